# revision 5
# baseline (speedup 1.0000x reference)
"""ClockworkRNN forward kernel for 8 Trainium2 NeuronCores.

Strategy: data-parallel over batch (64 -> 8 per core).  Everything on-chip is
kept "d-major": the recurrent state H lives as [128 partitions(d within
group), 8 groups * 8 batch] so the per-step tanh is one cheap activation and
the clock matmuls use the natural cw layout as stationary weights.

Per core:
  - projection x = X @ W + b computed with bf16 matmuls (W chunks stationary,
    X^T streamed), scattered into a per-step record buffer xrec[:, t*64+g*8+b]
  - 512-step scan; step t updates groups 0..m where m = min(trailing zeros of
    t, 7).  xt is injected into PSUM with an identity matmul (start=True) and
    clock matmuls accumulate on top; tanh of the active groups reads PSUM.
    Inactive (carried) groups skip the PE entirely: h_new = tanh(h_prev) runs
    as a second SBUF->SBUF activation emitted BEFORE the active-group tanh,
    so the in-order ACT engine does the carried work inside the PE/sem wait
    window and the next step's PE unblocks on the small active-group tanh.
  - output written to DRAM in scratch layout [128(dg), T, 8(g)*8(b)] as
    bf16 (h is bf16-quantized anyway); the host reshapes/upcasts to
    [B, T, D] fp32 (free - not on the device clock).

Performance: the kernel is latency-bound, not bandwidth/compute-bound: the
512 sequential steps each cost ~2 cross-engine semaphore hops (~100ns sem
propagation each) + one fixed-cost tanh activation (~240ns) + a burst of
tiny matmuls.  TimelineSim cost model: ~390us; measured on HW via repeat-R
slope: ~0.25-0.35ms.  Projection is interleaved into the scan's stall
windows in 64-step blocks, so it adds almost nothing to the critical path.
Accuracy vs the fp32 reference: rel l2 error ~4.6e-3 (bf16 state/weights).
"""

import sys

if "/opt/trn_rl_repo" not in sys.path:
    sys.path.insert(0, "/opt/trn_rl_repo")

import numpy as np
import ml_dtypes

import concourse.tile as tile
from concourse import bacc, mybir
from concourse import bass_utils
from concourse.masks import make_identity

BF16 = ml_dtypes.bfloat16
N_CORES = 8
B, T, IN, D = 64, 512, 512, 1024
N = 128          # units per clock group
G = 8            # number of clock groups
BL = B // N_CORES  # batch per core
KCH = IN // 128  # contraction chunks for the projection

_CACHE = {}


def _m_of(t: int) -> int:
    """Highest active group index at step t (prefix 0..m updates)."""
    if t == 0:
        return G - 1
    return min((t & -t).bit_length() - 1, G - 1)


def _pair(i: int, k: int) -> int:
    """Index of chunk k of cw_i in the packed CW buffer."""
    return i * (i + 1) // 2 + k


def build_nc(repeats: int = 1):
    nc = bacc.Bacc("TRN2", target_bir_lowering=False, debug=False,
                   num_devices=N_CORES)

    XT = nc.dram_tensor("XT", [IN, BL * T], mybir.dt.bfloat16,
                        kind="ExternalInput")
    Wt = nc.dram_tensor("Wt", [IN, D], mybir.dt.bfloat16,
                        kind="ExternalInput")
    CW = nc.dram_tensor("CW", [N, 36 * N], mybir.dt.bfloat16,
                        kind="ExternalInput")
    BIAS = nc.dram_tensor("BIAS", [N, G], mybir.dt.float32,
                          kind="ExternalInput")
    OUT = nc.dram_tensor("OUT", [N, T, G * BL], mybir.dt.bfloat16,
                         kind="ExternalOutput")

    f32 = mybir.dt.float32
    bf16 = mybir.dt.bfloat16
    Tanh = mybir.ActivationFunctionType.Tanh

    with tile.TileContext(nc) as tc:
        with (
            tc.tile_pool(name="const", bufs=1) as const,
            tc.tile_pool(name="hpool", bufs=6) as hpool,
            tc.tile_pool(name="projp", bufs=2, space="PSUM") as ppool,
            tc.tile_pool(name="scanp", bufs=4, space="PSUM") as pspool,
        ):
            # ---- persistent SBUF state ----
            xt_sb = const.tile([128, KCH, BL * T], bf16)     # X^T
            w_sb = const.tile([128, KCH, D], bf16)           # W chunks
            cw_sb = const.tile([128, 36 * N], bf16)          # packed cw chunks
            bias_sb = const.tile([128, G], f32)
            ident = const.tile([128, 128], bf16)
            xrec = const.tile([128, T * G * BL], bf16)       # per-step records

            # X^T arrives in TB-step blocks (col = (t//TB)*8*TB + b*TB + t%TB)
            xt_dram = XT.rearrange("(k p) c -> p k c", p=128)
            nc.sync.dma_start(out=xt_sb[:, :, 0:256],
                              in_=xt_dram[:, :, 0:256])  # block 0 first
            nc.sync.dma_start(out=w_sb,
                              in_=Wt.rearrange("(k p) d -> p k d", p=128))
            nc.sync.dma_start(out=bias_sb, in_=BIAS[:, :])
            nc.sync.dma_start(out=cw_sb, in_=CW[:, :])
            make_identity(nc, ident)

            TB = 32                    # timesteps per projection block
            NB = T // TB

            def proj_t0():
                """Seed xrec record 0 (t=0, all groups) so the scan can
                start while block 0's full projection is still running."""
                psum = ppool.tile([128, G * BL], f32, tag="proj0")
                xt_v = xt_sb.rearrange(
                    "p k (jj b tin) -> p k jj b tin", jj=NB, b=BL)
                rhs0 = xt_v[:, :, 0, :, 0]          # [p, k, b]
                for g in range(G):
                    for k in range(KCH):
                        nc.tensor.matmul(
                            psum[:, g * BL:(g + 1) * BL],
                            lhsT=w_sb[:, k, g * N:(g + 1) * N],
                            rhs=rhs0[:, k],
                            start=(g == 0 and k == 0), stop=(k == KCH - 1),
                            skip_group_check=True)
                xr_v = xrec.rearrange("p (t g b) -> p t g b", g=G, b=BL)
                for g in range(G):
                    nc.vector.tensor_scalar_add(
                        out=xr_v[:, 0, g, :], in0=psum[:, g * BL:(g + 1) * BL],
                        scalar1=bias_sb[:, g:g + 1])

            def proj_block(j, with_dma=True):
                """Project x for timesteps [j*TB, (j+1)*TB) into xrec."""
                if with_dma:
                    nc.sync.dma_start(
                        out=xt_sb[:, :, j * 8 * TB:(j + 1) * 8 * TB],
                        in_=xt_dram[:, :, j * 8 * TB:(j + 1) * 8 * TB])
                for g in range(G):
                    s = 1 << g
                    if s > TB:         # group 7: period 128 = 2 blocks
                        if j % (s // TB):
                            continue
                        ntin = 1
                        xt_v = xt_sb.rearrange(
                            "p k (jj b tin) -> p k jj b tin", jj=NB, b=BL)
                        rhs = xt_v[:, :, j, :, 0]       # [p, k, b]
                        xr_v = xrec.rearrange(
                            "p (jj tin g b) -> p jj tin g b",
                            jj=NB, tin=TB, g=G)
                        dest = xr_v[:, j, 0, g, :]      # [p, b]
                    else:
                        ntin = TB // s
                        xt_v = xt_sb.rearrange(
                            "p k (jj b tq ss) -> p k jj b tq ss",
                            jj=NB, b=BL, ss=s)
                        rhs = xt_v[:, :, j, :, :, 0]    # [p, k, b, tq]
                        xr_v = xrec.rearrange(
                            "p (jj tq ss g b) -> p jj tq ss g b",
                            jj=NB, ss=s, g=G, b=BL)
                        dest = xr_v[:, j, :, 0, g, :].rearrange(
                            "p t b -> p b t")           # [p, b, tq]
                    cols = BL * ntin
                    psum = ppool.tile([128, 512], f32, tag="proj")
                    pv = psum[:, :cols].rearrange("p (b t) -> p b t", b=BL)
                    for k in range(KCH):
                        nc.tensor.matmul(
                            pv, lhsT=w_sb[:, k, g * N:(g + 1) * N],
                            rhs=rhs[:, k],
                            start=(k == 0), stop=(k == KCH - 1),
                        )
                    nc.vector.tensor_scalar_add(
                        out=dest, in0=pv if ntin > 1 else pv[:, :, 0],
                        scalar1=bias_sb[:, g:g + 1],
                    )

            def body():
                # scan.  H lives in 8-step staging tiles so the tanh output
                # doubles as the DMA source (one 64KB store per 8 steps).
                # Projection for block j+1 is emitted just after block j's
                # first step so it executes inside the scan's stall windows.
                proj_t0()
                h0 = hpool.tile([128, G * BL], bf16, tag="H0")
                nc.vector.memset(h0, 0.0)
                h_prev = h0

                stg = None
                for t in range(T):
                    if t == 1:
                        proj_block(0, with_dma=False)
                    if t % TB == 2 and t // TB + 1 < NB:
                        proj_block(t // TB + 1)
                    m = _m_of(t)
                    act = BL * (m + 1)
                    ps = pspool.tile([128, G * BL], f32, tag="ps")

                    # xt -> psum (identity matmul; start=True clears the
                    # bank's has_written bits so clock matmuls accumulate).
                    nc.tensor.matmul(
                        ps[:, 0:act], lhsT=ident,
                        rhs=xrec[:, t * G * BL: t * G * BL + act],
                        start=True, stop=False, skip_group_check=True,
                    )
                    # clock matmuls accumulate
                    for i in range(m + 1):
                        for k in range(i + 1):
                            p = _pair(i, k)
                            nc.tensor.matmul(
                                ps[:, BL * i: BL * (i + 1)],
                                lhsT=cw_sb[:, p * N:(p + 1) * N],
                                rhs=h_prev[:, BL * k: BL * (k + 1)],
                                start=False, stop=(k == i),
                                skip_group_check=True,
                            )

                    if t % 8 == 0:
                        stg = hpool.tile([128, 8, G * BL], bf16, tag="stg")
                    h_new = stg[:, t % 8, :]
                    # Carried groups need no PE round trip: h_new = tanh(h_prev)
                    # straight from SBUF.  Emitted FIRST so the in-order ACT
                    # engine does this work inside the PE/sem wait window; the
                    # next step's PE then unblocks on the small active-group
                    # tanh below.
                    if m < G - 1:
                        nc.scalar.activation(
                            h_new[:, act:], h_prev[:, act:], Tanh)
                    nc.scalar.activation(h_new[:, 0:act], ps[:, 0:act], Tanh)
                    if t % 8 == 7:
                        nc.sync.dma_start(out=OUT[:, t - 7:t + 1, :], in_=stg)

                    h_prev = h_new

            for _rep in range(repeats):
                body()

    nc.compile()
    return nc


def _prep_in_maps(X, W, b, cws):
    cw_pack = np.concatenate(
        [cws[i][k * N:(k + 1) * N, :] for i in range(G) for k in range(i + 1)],
        axis=1).astype(BF16)                       # [128, 4608]
    w_in = W.astype(BF16)
    bias_in = np.ascontiguousarray(b.reshape(G, N).T.astype(np.float32))
    in_maps = []
    for c in range(N_CORES):
        xc = X[c * BL:(c + 1) * BL]                # [BL, T, IN]
        # col layout: (t//TB)*8*TB + b*TB + t%TB with TB=32
        xt_in = np.ascontiguousarray(
            xc.transpose(2, 0, 1).reshape(IN, BL, T // 32, 32)
            .transpose(0, 2, 1, 3).reshape(IN, BL * T)).astype(BF16)
        in_maps.append({
            "XT": xt_in, "Wt": w_in, "CW": cw_pack, "BIAS": bias_in,
        })
    return in_maps


def _assemble(results):
    out = np.empty((B, T, D), np.float32)
    for c in range(N_CORES):
        o = results[c]["OUT"].astype(np.float32)   # [128, T, 64] bf16
        out[c * BL:(c + 1) * BL] = (
            o.reshape(N, T, G, BL).transpose(3, 1, 2, 0).reshape(BL, T, D))
    return out


def kernel(X, W, b, cw0, cw1, cw2, cw3, cw4, cw5, cw6, cw7):
    X = np.asarray(X, np.float32)
    W = np.asarray(W, np.float32)
    b = np.asarray(b, np.float32)
    cws = [np.asarray(c, np.float32)
           for c in (cw0, cw1, cw2, cw3, cw4, cw5, cw6, cw7)]

    if "nc" not in _CACHE:
        _CACHE["nc"] = build_nc()
    nc = _CACHE["nc"]

    in_maps = _prep_in_maps(X, W, b, cws)
    res = bass_utils.run_bass_kernel_spmd(
        nc, in_maps, core_ids=list(range(N_CORES)))
    return _assemble(res.results)



# revision 8
# speedup vs baseline: 6683.3862x; 6683.3862x over previous
"""ClockworkRNN forward kernel for 8 Trainium2 NeuronCores.

Strategy: data-parallel over batch (64 -> 8 per core).  Everything on-chip is
kept "d-major": the recurrent state H lives as [128 partitions(d within
group), 8 groups * 8 batch] so the per-step tanh is one cheap activation and
the clock matmuls use the natural cw layout as stationary weights.

Per core:
  - projection x = X @ W + b computed with bf16 matmuls (W chunks stationary,
    X^T streamed), scattered into a per-step record buffer xrec[:, t*64+g*8+b]
  - 512-step scan; step t updates groups 0..m where m = min(trailing zeros of
    t, 7).  xt is injected into PSUM with an identity matmul (start=True) and
    clock matmuls accumulate on top; tanh of the active groups reads PSUM.
    Inactive (carried) groups skip the PE entirely: h_new = tanh(h_prev) runs
    as a second SBUF->SBUF activation emitted BEFORE the active-group tanh,
    so the in-order ACT engine does the carried work inside the PE/sem wait
    window and the next step's PE unblocks on the small active-group tanh.
  - output written to DRAM in scratch layout [128(dg), T, 8(g)*8(b)] as
    bf16 (h is bf16-quantized anyway); the host reshapes/upcasts to
    [B, T, D] fp32 (free - not on the device clock).

Performance: the kernel is latency-bound, not bandwidth/compute-bound: the
512 sequential steps each cost PE->ACT->PE semaphore round trips (~100ns
propagation each way) + the active-group tanh's exec+write-ack (~400ns,
init-cycle dominated).  Keeping the carried groups off the PE (direct
SBUF->SBUF tanh in the wait window) shrinks the PSUM tanh to the active
prefix and removes the identity carry matmul: TimelineSim 388.5us -> 353.7us
vs the previous formulation.  Projection is interleaved into the scan's
stall windows in 64-step blocks, so it adds almost nothing to the critical
path.  Per-step cost is within ~5% of the structural floor for a
batch-sharded sequential scan on this sync model (2 sem hops + 1 ACT
instruction minimum); going materially faster requires time-sharding the
scan across cores with speculative warmup, whose slow-clock groups
(periods 16-128) then need an exact cross-core event chain - not attempted.
Accuracy vs the fp32 reference: rel l2 error ~4.6e-3 (bf16 state/weights).
"""

import sys

if "/opt/trn_rl_repo" not in sys.path:
    sys.path.insert(0, "/opt/trn_rl_repo")

import numpy as np
import ml_dtypes

import concourse.tile as tile
from concourse import bacc, mybir
from concourse import bass_utils
from concourse.masks import make_identity

BF16 = ml_dtypes.bfloat16
N_CORES = 8
B, T, IN, D = 64, 512, 512, 1024
N = 128          # units per clock group
G = 8            # number of clock groups
BL = B // N_CORES  # batch per core
KCH = IN // 128  # contraction chunks for the projection

_CACHE = {}


def _m_of(t: int) -> int:
    """Highest active group index at step t (prefix 0..m updates)."""
    if t == 0:
        return G - 1
    return min((t & -t).bit_length() - 1, G - 1)


def _pair(i: int, k: int) -> int:
    """Index of chunk k of cw_i in the packed CW buffer."""
    return i * (i + 1) // 2 + k


def build_nc(repeats: int = 1):
    nc = bacc.Bacc("TRN2", target_bir_lowering=False, debug=False,
                   num_devices=N_CORES)

    XT = nc.dram_tensor("XT", [IN, BL * T], mybir.dt.bfloat16,
                        kind="ExternalInput")
    Wt = nc.dram_tensor("Wt", [IN, D], mybir.dt.bfloat16,
                        kind="ExternalInput")
    CW = nc.dram_tensor("CW", [N, 36 * N], mybir.dt.bfloat16,
                        kind="ExternalInput")
    BIAS = nc.dram_tensor("BIAS", [N, G], mybir.dt.float32,
                          kind="ExternalInput")
    OUT = nc.dram_tensor("OUT", [N, T, G * BL], mybir.dt.bfloat16,
                         kind="ExternalOutput")

    f32 = mybir.dt.float32
    bf16 = mybir.dt.bfloat16
    Tanh = mybir.ActivationFunctionType.Tanh

    with tile.TileContext(nc) as tc:
        with (
            tc.tile_pool(name="const", bufs=1) as const,
            tc.tile_pool(name="hpool", bufs=8) as hpool,
            tc.tile_pool(name="projp", bufs=2, space="PSUM") as ppool,
            tc.tile_pool(name="scanp", bufs=4, space="PSUM") as pspool,
        ):
            # ---- persistent SBUF state ----
            xt_sb = const.tile([128, KCH, BL * T], bf16)     # X^T
            w_sb = const.tile([128, KCH, D], bf16)           # W chunks
            cw_sb = const.tile([128, 36 * N], bf16)          # packed cw chunks
            bias_sb = const.tile([128, G], f32)
            ident = const.tile([128, 128], bf16)
            xrec = const.tile([128, T * G * BL], bf16)       # per-step records

            # X^T arrives in TB-step blocks (col = (t//TB)*8*TB + b*TB + t%TB)
            xt_dram = XT.rearrange("(k p) c -> p k c", p=128)
            nc.sync.dma_start(out=xt_sb[:, :, 0:256],
                              in_=xt_dram[:, :, 0:256])  # block 0 first
            nc.sync.dma_start(out=w_sb,
                              in_=Wt.rearrange("(k p) d -> p k d", p=128))
            nc.sync.dma_start(out=bias_sb, in_=BIAS[:, :])
            nc.sync.dma_start(out=cw_sb, in_=CW[:, :])
            make_identity(nc, ident)

            TB = 32                    # timesteps per projection block
            NB = T // TB

            def proj_t0():
                """Seed xrec record 0 (t=0, all groups) so the scan can
                start while block 0's full projection is still running."""
                psum = ppool.tile([128, G * BL], f32, tag="proj0")
                xt_v = xt_sb.rearrange(
                    "p k (jj b tin) -> p k jj b tin", jj=NB, b=BL)
                rhs0 = xt_v[:, :, 0, :, 0]          # [p, k, b]
                for g in range(G):
                    for k in range(KCH):
                        nc.tensor.matmul(
                            psum[:, g * BL:(g + 1) * BL],
                            lhsT=w_sb[:, k, g * N:(g + 1) * N],
                            rhs=rhs0[:, k],
                            start=(g == 0 and k == 0), stop=(k == KCH - 1),
                            skip_group_check=True)
                xr_v = xrec.rearrange("p (t g b) -> p t g b", g=G, b=BL)
                for g in range(G):
                    nc.vector.tensor_scalar_add(
                        out=xr_v[:, 0, g, :], in0=psum[:, g * BL:(g + 1) * BL],
                        scalar1=bias_sb[:, g:g + 1])

            def proj_block(j, with_dma=True):
                """Project x for timesteps [j*TB, (j+1)*TB) into xrec."""
                if with_dma:
                    nc.sync.dma_start(
                        out=xt_sb[:, :, j * 8 * TB:(j + 1) * 8 * TB],
                        in_=xt_dram[:, :, j * 8 * TB:(j + 1) * 8 * TB])
                for g in range(G):
                    s = 1 << g
                    if s > TB:         # group 7: period 128 = 2 blocks
                        if j % (s // TB):
                            continue
                        ntin = 1
                        xt_v = xt_sb.rearrange(
                            "p k (jj b tin) -> p k jj b tin", jj=NB, b=BL)
                        rhs = xt_v[:, :, j, :, 0]       # [p, k, b]
                        xr_v = xrec.rearrange(
                            "p (jj tin g b) -> p jj tin g b",
                            jj=NB, tin=TB, g=G)
                        dest = xr_v[:, j, 0, g, :]      # [p, b]
                    else:
                        ntin = TB // s
                        xt_v = xt_sb.rearrange(
                            "p k (jj b tq ss) -> p k jj b tq ss",
                            jj=NB, b=BL, ss=s)
                        rhs = xt_v[:, :, j, :, :, 0]    # [p, k, b, tq]
                        xr_v = xrec.rearrange(
                            "p (jj tq ss g b) -> p jj tq ss g b",
                            jj=NB, ss=s, g=G, b=BL)
                        dest = xr_v[:, j, :, 0, g, :].rearrange(
                            "p t b -> p b t")           # [p, b, tq]
                    cols = BL * ntin
                    psum = ppool.tile([128, 512], f32, tag="proj")
                    pv = psum[:, :cols].rearrange("p (b t) -> p b t", b=BL)
                    for k in range(KCH):
                        nc.tensor.matmul(
                            pv, lhsT=w_sb[:, k, g * N:(g + 1) * N],
                            rhs=rhs[:, k],
                            start=(k == 0), stop=(k == KCH - 1),
                        )
                    nc.vector.tensor_scalar_add(
                        out=dest, in0=pv if ntin > 1 else pv[:, :, 0],
                        scalar1=bias_sb[:, g:g + 1],
                    )

            def body():
                # scan.  H lives in 8-step staging tiles so the tanh output
                # doubles as the DMA source (one 64KB store per 8 steps).
                # Projection for block j+1 is emitted just after block j's
                # first step so it executes inside the scan's stall windows.
                proj_t0()
                h0 = hpool.tile([128, G * BL], bf16, tag="H0")
                nc.vector.memset(h0, 0.0)
                h_prev = h0

                stg = None
                for t in range(T):
                    if t == 1:
                        proj_block(0, with_dma=False)
                    if t % TB == 2 and t // TB + 1 < NB:
                        proj_block(t // TB + 1)
                    m = _m_of(t)
                    act = BL * (m + 1)
                    ps = pspool.tile([128, G * BL], f32, tag="ps")

                    # xt -> psum (identity matmul; start=True clears the
                    # bank's has_written bits so clock matmuls accumulate).
                    nc.tensor.matmul(
                        ps[:, 0:act], lhsT=ident,
                        rhs=xrec[:, t * G * BL: t * G * BL + act],
                        start=True, stop=False, skip_group_check=True,
                    )
                    # clock matmuls accumulate
                    for i in range(m + 1):
                        for k in range(i + 1):
                            p = _pair(i, k)
                            nc.tensor.matmul(
                                ps[:, BL * i: BL * (i + 1)],
                                lhsT=cw_sb[:, p * N:(p + 1) * N],
                                rhs=h_prev[:, BL * k: BL * (k + 1)],
                                start=False, stop=(k == i),
                                skip_group_check=True,
                            )

                    if t % 8 == 0:
                        stg = hpool.tile([128, 8, G * BL], bf16, tag="stg")
                    h_new = stg[:, t % 8, :]
                    # Carried groups need no PE round trip: h_new = tanh(h_prev)
                    # straight from SBUF.  Emitted FIRST so the in-order ACT
                    # engine does this work inside the PE/sem wait window; the
                    # next step's PE then unblocks on the small active-group
                    # tanh below.
                    if m < G - 1:
                        nc.scalar.activation(
                            h_new[:, act:], h_prev[:, act:], Tanh)
                    nc.scalar.activation(h_new[:, 0:act], ps[:, 0:act], Tanh)
                    if t % 8 == 7:
                        nc.sync.dma_start(out=OUT[:, t - 7:t + 1, :], in_=stg)

                    h_prev = h_new

            for _rep in range(repeats):
                body()

    nc.compile()
    return nc


def _prep_in_maps(X, W, b, cws):
    cw_pack = np.concatenate(
        [cws[i][k * N:(k + 1) * N, :] for i in range(G) for k in range(i + 1)],
        axis=1).astype(BF16)                       # [128, 4608]
    w_in = W.astype(BF16)
    bias_in = np.ascontiguousarray(b.reshape(G, N).T.astype(np.float32))
    in_maps = []
    for c in range(N_CORES):
        xc = X[c * BL:(c + 1) * BL]                # [BL, T, IN]
        # col layout: (t//TB)*8*TB + b*TB + t%TB with TB=32
        xt_in = np.ascontiguousarray(
            xc.transpose(2, 0, 1).reshape(IN, BL, T // 32, 32)
            .transpose(0, 2, 1, 3).reshape(IN, BL * T)).astype(BF16)
        in_maps.append({
            "XT": xt_in, "Wt": w_in, "CW": cw_pack, "BIAS": bias_in,
        })
    return in_maps


def _assemble(results):
    out = np.empty((B, T, D), np.float32)
    for c in range(N_CORES):
        o = results[c]["OUT"].astype(np.float32)   # [128, T, 64] bf16
        out[c * BL:(c + 1) * BL] = (
            o.reshape(N, T, G, BL).transpose(3, 1, 2, 0).reshape(BL, T, D))
    return out


def kernel(X, W, b, cw0, cw1, cw2, cw3, cw4, cw5, cw6, cw7):
    X = np.asarray(X, np.float32)
    W = np.asarray(W, np.float32)
    b = np.asarray(b, np.float32)
    cws = [np.asarray(c, np.float32)
           for c in (cw0, cw1, cw2, cw3, cw4, cw5, cw6, cw7)]

    if "nc" not in _CACHE:
        _CACHE["nc"] = build_nc()
    nc = _CACHE["nc"]

    in_maps = _prep_in_maps(X, W, b, cws)
    res = bass_utils.run_bass_kernel_spmd(
        nc, in_maps, core_ids=list(range(N_CORES)))
    return _assemble(res.results)

